# revision 21
# baseline (speedup 1.0000x reference)
"""Trainium2 Bass kernel for nn_MetricSelfAttention.

Math: the reference's softmax is dead code, so
    nudged = (p1 @ M @ p2^T) @ p1
reassociates to
    nudged = p1 @ (M @ (p2^T @ p1))        (per-head 64x64 Gram matrix G)
collapsing the O(W^2) attention matrices entirely, and the mixer folds to
    out = p1u @ Whm,   Whm = H_bd @ Wmix_slice   (precomputed once on-chip)
so the back end is a single matmul per w-tile straight from p1^T.

Sharding: 8 cores = 2 batches x 4 head-pairs.  Core (b, hg) computes heads
{2hg, 2hg+1} of batch b and the partial output; the host sums the 4 partials
per batch and adds b_mixer.  x1 arrives pre-transposed, everything pre-cast
to bf16 on the host (layout/dtype prep only).

LayerNorm folding (no normalized tensors are materialized):
  - gamma folds into the projection on the host; nonzero beta enters as
    rank-1 bias matmuls; omitted entirely when beta == 0.
  - x1 row stats come from x1^T via PE ones-matmuls producing rows at
    partitions 0/32 of one PSUM bank, which are PE-transposed to columns
    ([33,128] transposes) so all variance/sqrt/reciprocal arithmetic runs
    128-lane parallel.  Row-form stats never touch DVE single-lane ops.
  - rstd1 never exists in row form: p1^T stays UNSCALED; rstd1*rstd2 scales
    the transposed p1s columns (0-stride-broadcast paired multiply), and
    rstd1 alone scales the output-stage PSUM evacuation (per-partition).
    Mean-subtraction enters the projection as one rank-1 matmul
    (-colsum) (x) mu_row with colsum negated on the host.
  - x2 is never normalized or transposed:
      F := x2n^T @ p1  ==  x2^T @ p1s - 1 (x) (mu2^T @ p1s),  p1s = rstd12*p1u

All tensors flow as bf16 (PSUM accumulation in fp32); partial outputs are
written bf16 and summed in fp32 on the host.
"""

from contextlib import ExitStack

import numpy as np
import ml_dtypes

import concourse.bacc as bacc
import concourse.bass as bass
import concourse.tile as tile
from concourse import mybir
from concourse.bass_utils import run_bass_kernel_spmd
from concourse.masks import make_identity

B, W, C, N, K = 2, 2048, 512, 8, 64
NCORES = 8
HPC = 2          # heads per core
K2 = HPC * K     # 128 channels per core
EPS = 1e-5
FP32 = mybir.dt.float32
BF16 = mybir.dt.bfloat16
NPBF16 = ml_dtypes.bfloat16
AF = mybir.ActivationFunctionType

NT = W // 128    # 16 w-tiles
NQ = W // 512    # 4 w-quads
NJ = C // 128    # 4 c-chunks


def _free_bcast(ap2d, n):
    """[128, m] AP -> [128, m, n] view with 0-stride innermost dim."""
    return bass.AP(tensor=ap2d.tensor, offset=ap2d.offset,
                   ap=list(ap2d.ap) + [[0, n]])


def _body(ctx: ExitStack, tc: tile.TileContext, x1td, x2d, projd, mmatd,
          wmixd, colsumd, outd, pbiasrd):
    nc = tc.nc
    with_pbias = pbiasrd is not None

    persist = ctx.enter_context(tc.tile_pool(name="persist", bufs=1))
    sqpool = ctx.enter_context(tc.tile_pool(name="sq", bufs=2))
    rowpool = ctx.enter_context(tc.tile_pool(name="rows", bufs=2))
    spool = ctx.enter_context(tc.tile_pool(name="stats", bufs=4))
    outpool = ctx.enter_context(tc.tile_pool(name="outstage", bufs=2))
    ps_tp = ctx.enter_context(tc.tile_pool(
        name="ps_tp", bufs=1 if with_pbias else 2, space="PSUM"))
    ps_acc = ctx.enter_context(tc.tile_pool(name="ps_acc", bufs=1, space="PSUM"))
    ps_st = ctx.enter_context(tc.tile_pool(name="ps_st", bufs=1, space="PSUM"))
    ps_mm = ctx.enter_context(tc.tile_pool(name="ps_mm", bufs=2, space="PSUM"))
    ps_mo = ctx.enter_context(tc.tile_pool(name="ps_mo", bufs=2, space="PSUM"))

    # ---- persistent activations (declared first; loads issued immediately) -
    x1t_r = persist.tile([128, NJ, W], BF16)    # x1^T
    x2r_s = persist.tile([128, NT, C], BF16)    # raw x2
    for q in range(NQ):
        qs = slice(q * 512, (q + 1) * 512)
        nc.gpsimd.dma_start(
            out=x1t_r[:, :, qs],
            in_=x1td[:, qs].rearrange("(j p) w -> p j w", p=128))
        nc.gpsimd.dma_start(
            out=x2r_s[:, 4 * q:4 * (q + 1), :],
            in_=x2d[qs, :].rearrange("(t p) c -> p t c", p=128))

    # ---- constants / params (all bf16 from host) ---------------------------
    proj_s = persist.tile([128, NJ, K2], BF16)
    nc.sync.dma_start(out=proj_s, in_=projd.rearrange("(j p) k -> p j k", p=128))
    wmix_s = persist.tile([K2, C], BF16)
    nc.sync.dma_start(out=wmix_s, in_=wmixd)
    mmat_s = persist.tile([K, HPC, K], BF16)
    nc.sync.dma_start(out=mmat_s, in_=mmatd)
    ncolsum_s = persist.tile([1, K2], BF16)     # NEGATED colsum (host)
    nc.sync.dma_start(out=ncolsum_s, in_=colsumd)
    pbiasr_s = None
    if with_pbias:
        pbiasr_s = persist.tile([1, K2], BF16)
        nc.sync.dma_start(out=pbiasr_s, in_=pbiasrd)

    eps_s = persist.tile([128, 1], FP32)
    nc.vector.memset(eps_s, EPS)
    oneshalf = persist.tile([128, 1], BF16)
    nc.vector.memset(oneshalf, 1.0 / C)
    ident = persist.tile([128, 128], BF16)
    nc.vector.memset(ident, 0.0)
    make_identity(nc, ident, nomemset=True)
    # PE warm-up: data-independent matmuls while input DMA streams, so the
    # tensor engine p-state ramps before real work arrives.
    warm = ps_mo.tile([128, 512], FP32, tag="mo")
    for i in range(56):
        nc.tensor.matmul(warm[:, :128], lhsT=ident, rhs=ident,
                         start=True, stop=True)

    # ---- persistent activations --------------------------------------------
    rstd2_s = persist.tile([128, NT], FP32)     # per-row 1/std of x2
    mu2_s = persist.tile([128, NT], BF16)       # per-row NEGATED mean of x2
    rstd1_s = persist.tile([128, NT], FP32)     # per-row 1/std of x1
    sc12_s = persist.tile([128, NT], FP32)      # rstd1*rstd2 per row
    p1s_s = persist.tile([128, NT, K2], BF16)   # rstd1*rstd2 * p1u (rowwise)
    p1T_s = persist.tile([K2, W], BF16)         # p1^T UNSCALED (p1u^T)
    ft_s = persist.tile([K2, C], BF16)          # F^T
    f_s = persist.tile([128, NJ, K2], BF16)     # F (c on partitions)
    g_s = persist.tile([K, HPC, K], BF16)       # per-head Gram
    ht_bd_s = persist.tile([K2, K2], BF16)      # block-diag H^T = (M G)^T
    whm_s = persist.tile([K2, C], BF16)         # Whm = H_bd @ Wmix_slice
    if with_pbias:
        std2_s = persist.tile([128, NT], BF16)  # per-row std of x2
        s1_s = persist.tile([1, K2], BF16)      # column sums of p1

    def p1s_pairs(q):
        # paired PE-transposes + 0-stride-broadcast scale for quad q
        for tp in range(2):
            w0 = 4 * q + 2 * tp
            ps2 = ps_tp.tile([128, 2, 128], BF16, tag="tp")
            for i in range(2):
                nc.tensor.transpose(
                    ps2[:, i, :],
                    p1T_s[:, (w0 + i) * 128:(w0 + i + 1) * 128], ident)
            nc.vector.tensor_mul(p1s_s[:, w0:w0 + 2, :], ps2,
                                 _free_bcast(sc12_s[:, w0:w0 + 2], K2))

    facc = ps_acc.tile([128, 512], FP32, tag="facc")

    def wrow_f_partials(q):
        # F^T += p1s_t^T @ x2_t  and the folded mean-removal rank-1:
        # F^T += p1s_t^T @ ((-mu2_t) (x) ones) via 0-stride broadcast rhs.
        for t in range(4 * q, 4 * q + 4):
            nc.tensor.matmul(facc, lhsT=p1s_s[:, t, :], rhs=x2r_s[:, t, :],
                             start=(t == 0), stop=False)
        for t in range(4 * q, 4 * q + 4):
            nc.tensor.matmul(facc, lhsT=p1s_s[:, t, :],
                             rhs=_free_bcast(mu2_s[:, t:t + 1], 512),
                             start=False, stop=(t == NT - 1))

    # ========================================================================
    # Front end, pipelined per w-quad.
    # ========================================================================
    for q in range(NQ):
        qs = slice(q * 512, (q + 1) * 512)
        q4 = slice(4 * q, 4 * (q + 1))

        # ---- x1 squares on scalar (feed ssq matmuls) ----------------------
        sqt = sqpool.tile([128, NJ, 512], BF16, tag="sq")
        for j in range(NJ):
            nc.scalar.activation(sqt[:, j, :], x1t_r[:, j, qs], AF.Square)

        # ---- x2 row stats: bn per tile, tail ops batched over the quad ----
        mv = spool.tile([128, 4, 2], FP32, tag="mv")
        for t in range(4):
            stats = spool.tile([128, 6], FP32, tag="bst")
            nc.vector.bn_stats(stats, x2r_s[:, 4 * q + t, :])
            nc.vector.bn_aggr(mv[:, t, :], stats)
        std2q = spool.tile([128, 4], FP32, tag="stdq")
        nc.scalar.activation(std2q, mv[:, :, 1], AF.Sqrt, bias=eps_s, scale=1.0)
        nc.vector.reciprocal(rstd2_s[:, q4], std2q)
        nc.gpsimd.tensor_scalar_mul(mu2_s[:, q4], mv[:, :, 0], -1.0)
        if with_pbias:
            nc.gpsimd.tensor_copy(out=std2_s[:, q4], in_=std2q)

        # ---- PE: projection mains + mu stats (need only x1t) --------------
        pt = ps_mm.tile([128, 512], FP32, tag="mm")
        st_ps = ps_mo.tile([128, 512], FP32, tag="mo")
        mu_psv = st_ps[0:1, :]
        ssq_psv = st_ps[32:33, :]
        for j in range(NJ):
            nc.tensor.matmul(pt, lhsT=proj_s[:, j, :], rhs=x1t_r[:, j, qs],
                             start=(j == 0), stop=False)
        for j in range(NJ):
            nc.tensor.matmul(mu_psv, lhsT=oneshalf, rhs=x1t_r[:, j, qs],
                             start=(j == 0), stop=(j == NJ - 1))
        # filler: prev-quad transposes/partials while squares catch up
        if q > 0:
            p1s_pairs(q - 1)
            if q > 1:
                wrow_f_partials(q - 2)
        for j in range(NJ):
            nc.tensor.matmul(ssq_psv, lhsT=oneshalf, rhs=sqt[:, j, :],
                             start=(j == 0), stop=(j == NJ - 1))
        strow = rowpool.tile([33, 512], BF16, tag="strow")
        nc.scalar.copy(out=strow, in_=st_ps[0:33, :])

        # ---- close projection: += (-colsum) (x) mu_row --------------------
        nc.tensor.matmul(pt, lhsT=ncolsum_s, rhs=strow[0:1, :],
                         start=False, stop=not with_pbias)

        # ---- merged stat transposes: [33,128] -> [128,33] columns ---------
        stc = ps_st.tile([128, 4, 34], BF16, tag="st")
        for t in range(4):
            nc.tensor.transpose(stc[:, t, 0:33],
                                strow[:, t * 128:(t + 1) * 128],
                                ident[0:33, 0:33])
        stq = spool.tile([128, 4, 34], BF16, tag="stq")
        nc.vector.tensor_copy(out=stq, in_=stc)
        musq = spool.tile([128, 4], FP32, tag="musq")
        nc.gpsimd.tensor_mul(musq, stq[:, :, 0], stq[:, :, 0])
        varq = spool.tile([128, 4], FP32, tag="varq")
        nc.gpsimd.tensor_sub(varq, stq[:, :, 32], musq)
        std1q = spool.tile([128, 4], FP32, tag="std1")
        nc.scalar.activation(std1q, varq, AF.Sqrt, bias=eps_s, scale=1.0)
        nc.vector.reciprocal(rstd1_s[:, q4], std1q)
        nc.gpsimd.tensor_mul(sc12_s[:, q4], rstd1_s[:, q4], rstd2_s[:, q4])

        if with_pbias:
            # p1 = rstd1*p1u + 1 (x) pbias => fold as p1u += std1 (x) pbias
            std1q_b = spool.tile([128, 4], BF16, tag="s1qb")
            nc.gpsimd.tensor_copy(out=std1q_b, in_=std1q)
            s1r_ps = ps_st.tile([1, 512], BF16, tag="s1rp")
            for t in range(4):
                nc.tensor.transpose(s1r_ps[:, t * 128:(t + 1) * 128],
                                    std1q_b[:, t:t + 1], ident)
            std1row = rowpool.tile([1, 512], BF16, tag="s1r")
            nc.gpsimd.tensor_copy(out=std1row, in_=s1r_ps)
            nc.tensor.matmul(pt, lhsT=pbiasr_s, rhs=std1row,
                             start=False, stop=True)

        nc.vector.tensor_copy(out=p1T_s[:, qs], in_=pt)

    p1s_pairs(NQ - 1)
    wrow_f_partials(NQ - 2)
    wrow_f_partials(NQ - 1)

    # ========================================================================
    # Gram tail
    # ========================================================================
    nc.vector.tensor_copy(out=ft_s[:, 0:256], in_=facc[:, 0:256])
    nc.scalar.copy(out=ft_s[:, 256:512], in_=facc[:, 256:512])

    # F = PE-transpose of F^T (c on partitions), via tp pair tiles
    for jp in range(2):
        fjp = ps_tp.tile([128, 2, 128], BF16, tag="tp")
        for i in range(2):
            j = 2 * jp + i
            nc.tensor.transpose(fjp[:, i, :],
                                ft_s[:, j * 128:(j + 1) * 128], ident)
        if jp == 0:
            nc.vector.tensor_copy(out=f_s[:, 0:2, :], in_=fjp)
        else:
            nc.scalar.copy(out=f_s[:, 2:4, :], in_=fjp)

    # s1 = column sums of p1 = std2^T @ p1s (beta rank-1 term in G)
    if with_pbias:
        sp = ps_mm.tile([128, 512], FP32, tag="mm")
        spv = sp[:1, :K2]
        for t in range(NT):
            nc.tensor.matmul(spv, lhsT=std2_s[:, t:t + 1], rhs=p1s_s[:, t, :],
                             start=(t == 0), stop=(t == NT - 1))
        nc.vector.tensor_copy(out=s1_s, in_=spv)

    # Full P'^T @ F product; per-head Grams are its diagonal 64x64 blocks.
    gp = ps_mm.tile([128, 512], FP32, tag="mm")
    gpv = gp[:, :K2]
    for j in range(NJ):
        nc.tensor.matmul(gpv, lhsT=proj_s[:, j, :], rhs=f_s[:, j, :],
                         start=(j == 0),
                         stop=(j == NJ - 1) and not with_pbias)
    if with_pbias:
        # += pbias (x) s1 ; diagonal blocks get pbias_h (x) s1_h
        nc.tensor.matmul(gpv, lhsT=pbiasr_s, rhs=s1_s, start=False, stop=True)
    for h in range(HPC):
        nc.vector.tensor_copy(out=g_s[:, h, :],
                              in_=gpv[h * K:(h + 1) * K, h * K:(h + 1) * K])

    # H^T_h = G_h^T @ M_h (M symmetric); assemble block-diag H^T
    hp = ps_mm.tile([128, 512], FP32, tag="mm")
    for h in range(HPC):
        nc.tensor.matmul(hp[h * K:(h + 1) * K, :K], lhsT=g_s[:, h, :],
                         rhs=mmat_s[:, h, :])
    nc.vector.memset(ht_bd_s, 0.0)
    for h in range(HPC):
        nc.vector.tensor_copy(out=ht_bd_s[h * K:(h + 1) * K, h * K:(h + 1) * K],
                              in_=hp[h * K:(h + 1) * K, :K])

    # Whm = H_bd @ Wmix_slice  (lhsT = H^T_bd)
    whp = ps_mm.tile([128, 512], FP32, tag="mm")
    nc.tensor.matmul(whp, lhsT=ht_bd_s, rhs=wmix_s)
    nc.vector.tensor_copy(out=whm_s, in_=whp)

    # out = rstd1 * (p1u @ Whm), evacuated with per-partition rstd1 scale
    for h in range(2 * NQ):
        hs = slice(h * 256, (h + 1) * 256)
        stage = outpool.tile([128, 2, C], BF16, tag="ostage")
        for t in range(2):
            w_t = h * 2 + t
            mo = ps_mo.tile([128, 512], FP32, tag="mo")
            nc.tensor.matmul(mo, lhsT=p1T_s[:, w_t * 128:(w_t + 1) * 128],
                             rhs=whm_s)
            if t % 2 == 0:
                nc.vector.tensor_scalar_mul(stage[:, t, :], mo,
                                            rstd1_s[:, w_t:w_t + 1])
            else:
                nc.scalar.activation(stage[:, t, :], mo, AF.Copy,
                                     scale=rstd1_s[:, w_t:w_t + 1])
        nc.sync.dma_start(
            out=outd[hs, :].rearrange("(t p) c -> p t c", p=128),
            in_=stage)


_PROGRAM_CACHE = {}


def _get_program(with_pbias: bool):
    key = ("nc", with_pbias)
    if key in _PROGRAM_CACHE:
        return _PROGRAM_CACHE[key]
    nc = bacc.Bacc("TRN2", debug=False, num_devices=NCORES)
    x1td = nc.dram_tensor("x1t", [C, W], BF16, kind="ExternalInput").ap()
    x2d = nc.dram_tensor("x2", [W, C], BF16, kind="ExternalInput").ap()
    projd = nc.dram_tensor("proj", [C, K2], BF16, kind="ExternalInput").ap()
    mmatd = nc.dram_tensor("mmat", [K, HPC, K], BF16, kind="ExternalInput").ap()
    wmixd = nc.dram_tensor("wmix", [K2, C], BF16, kind="ExternalInput").ap()
    colsumd = nc.dram_tensor("colsum", [1, K2], BF16, kind="ExternalInput").ap()
    pbiasrd = None
    if with_pbias:
        pbiasrd = nc.dram_tensor("pbiasr", [1, K2], BF16, kind="ExternalInput").ap()
    outd = nc.dram_tensor("out", [W, C], BF16, kind="ExternalOutput").ap()
    with tile.TileContext(nc) as tc:
        with ExitStack() as ctx:
            _body(ctx, tc, x1td, x2d, projd, mmatd, wmixd, colsumd, outd,
                  pbiasrd)
    nc.compile()
    _PROGRAM_CACHE[key] = nc
    return nc


def _host_prep(inputs):
    x1 = np.asarray(inputs["x1"], np.float32)
    x2 = np.ascontiguousarray(np.asarray(inputs["x2"], np.float32))
    gamma = np.asarray(inputs["gamma"], np.float32)
    beta = np.asarray(inputs["beta"], np.float32)
    proj = np.asarray(inputs["proj_nck"], np.float32)
    halves = np.asarray(inputs["halves"], np.float32)
    diagonals = np.asarray(inputs["diagonals"], np.float32)
    wmix = np.asarray(inputs["W_mixer"], np.float32)

    iu0, iu1 = np.triu_indices(K, k=1)
    m = np.zeros((N, K, K), np.float32)
    m[:, iu0, iu1] = halves
    m = m + np.swapaxes(m, -1, -2)
    d = np.arange(K)
    m[:, d, d] = diagonals

    pgam = proj * gamma[None, :, None]          # gamma folded into projection
    with_pbias = bool(np.any(beta))
    pbias = np.einsum("c,nck->nk", beta, proj) if with_pbias else None

    x1t = [np.ascontiguousarray(x1[b].T).astype(NPBF16) for b in range(B)]
    x2b = [x2[b].astype(NPBF16) for b in range(B)]

    in_maps = []
    for core in range(NCORES):
        b, hg = divmod(core, NCORES // B)
        h0 = HPC * hg
        proj_core = np.ascontiguousarray(
            np.concatenate([pgam[h0 + i] for i in range(HPC)], axis=1))
        proj_bf = proj_core.astype(NPBF16)
        ncolsum = -proj_bf.astype(np.float32).sum(axis=0)
        im = {
            "x1t": x1t[b],
            "x2": x2b[b],
            "proj": proj_bf,
            "mmat": np.ascontiguousarray(
                np.stack([m[h0 + i] for i in range(HPC)], axis=1)).astype(NPBF16),
            "wmix": np.ascontiguousarray(
                wmix[:, K2 * hg:K2 * (hg + 1)].T).astype(NPBF16),
            "colsum": np.ascontiguousarray(ncolsum[None, :]).astype(NPBF16),
        }
        if with_pbias:
            pb = np.concatenate([pbias[h0 + i] for i in range(HPC)])
            im["pbiasr"] = np.ascontiguousarray(pb[None, :]).astype(NPBF16)
        in_maps.append(im)
    return in_maps, with_pbias


def kernel(**inputs) -> np.ndarray:
    in_maps, with_pbias = _host_prep(inputs)
    nc = _get_program(with_pbias)
    res = run_bass_kernel_spmd(nc, in_maps, core_ids=list(range(NCORES)))
    out = np.zeros((B, W, C), np.float32)
    for core in range(NCORES):
        b = core // (NCORES // B)
        out[b] += res.results[core]["out"].astype(np.float32)
    out += np.asarray(inputs["b_mixer"], np.float32)[None, None, :]
    return out


# revision 22
# speedup vs baseline: 1.0406x; 1.0406x over previous
"""Trainium2 Bass kernel for nn_MetricSelfAttention.

Math: the reference's softmax is dead code, so
    nudged = (p1 @ M @ p2^T) @ p1
reassociates to
    nudged = p1 @ (M @ (p2^T @ p1))        (per-head 64x64 Gram matrix G)
collapsing the O(W^2) attention matrices entirely, and the mixer folds to
    out = p1u @ Whm,   Whm = H_bd @ Wmix_slice   (precomputed once on-chip)
so the back end is a single matmul per w-tile straight from p1^T.

Sharding: 8 cores = 2 batches x 4 head-pairs.  Core (b, hg) computes heads
{2hg, 2hg+1} of batch b and the partial output; the host sums the 4 partials
per batch and adds b_mixer.  x1 arrives pre-transposed, everything pre-cast
to bf16 on the host (layout/dtype prep only).

LayerNorm folding (no normalized tensors are materialized):
  - gamma folds into the projection on the host; nonzero beta enters as
    rank-1 bias matmuls; omitted entirely when beta == 0.
  - x1 row stats come from x1^T via PE ones-matmuls producing rows at
    partitions 0/32 of one PSUM bank, which are PE-transposed to columns
    ([33,128] transposes) so all variance/sqrt/reciprocal arithmetic runs
    128-lane parallel.  Row-form stats never touch DVE single-lane ops.
  - rstd1 never exists in row form: p1^T stays UNSCALED; rstd1*rstd2 scales
    the transposed p1s columns (0-stride-broadcast paired multiply), and
    rstd1 alone scales the output-stage PSUM evacuation (per-partition).
    Mean-subtraction enters the projection as one rank-1 matmul
    (-colsum) (x) mu_row with colsum negated on the host.
  - x2 is never normalized or transposed:
      F := x2n^T @ p1  ==  x2^T @ p1s - 1 (x) (mu2^T @ p1s),  p1s = rstd12*p1u

All tensors flow as bf16 (PSUM accumulation in fp32); partial outputs are
written bf16 and summed in fp32 on the host.
"""

from contextlib import ExitStack

import numpy as np
import ml_dtypes

import concourse.bacc as bacc
import concourse.bass as bass
import concourse.tile as tile
from concourse import mybir
from concourse.bass_utils import run_bass_kernel_spmd
from concourse.masks import make_identity

B, W, C, N, K = 2, 2048, 512, 8, 64
NCORES = 8
HPC = 2          # heads per core
K2 = HPC * K     # 128 channels per core
EPS = 1e-5
FP32 = mybir.dt.float32
BF16 = mybir.dt.bfloat16
NPBF16 = ml_dtypes.bfloat16
AF = mybir.ActivationFunctionType

NT = W // 128    # 16 w-tiles
NQ = W // 512    # 4 w-quads
NJ = C // 128    # 4 c-chunks


def _free_bcast(ap2d, n):
    """[128, m] AP -> [128, m, n] view with 0-stride innermost dim."""
    return bass.AP(tensor=ap2d.tensor, offset=ap2d.offset,
                   ap=list(ap2d.ap) + [[0, n]])


def _body(ctx: ExitStack, tc: tile.TileContext, x1td, x2d, projd, mmatd,
          wmixd, colsumd, outd, pbiasrd):
    nc = tc.nc
    with_pbias = pbiasrd is not None

    persist = ctx.enter_context(tc.tile_pool(name="persist", bufs=1))
    sqpool = ctx.enter_context(tc.tile_pool(name="sq", bufs=2))
    rowpool = ctx.enter_context(tc.tile_pool(name="rows", bufs=2))
    spool = ctx.enter_context(tc.tile_pool(name="stats", bufs=4))
    outpool = ctx.enter_context(tc.tile_pool(name="outstage", bufs=2))
    ps_tp = ctx.enter_context(tc.tile_pool(
        name="ps_tp", bufs=1 if with_pbias else 2, space="PSUM"))
    ps_acc = ctx.enter_context(tc.tile_pool(name="ps_acc", bufs=1, space="PSUM"))
    ps_st = ctx.enter_context(tc.tile_pool(name="ps_st", bufs=1, space="PSUM"))
    ps_mm = ctx.enter_context(tc.tile_pool(name="ps_mm", bufs=2, space="PSUM"))
    ps_mo = ctx.enter_context(tc.tile_pool(name="ps_mo", bufs=2, space="PSUM"))

    # ---- persistent activations (declared first; loads issued immediately) -
    x1t_r = persist.tile([128, NJ, W], BF16)    # x1^T
    x2r_s = persist.tile([128, NT, C], BF16)    # raw x2
    for q in range(NQ):
        qs = slice(q * 512, (q + 1) * 512)
        nc.gpsimd.dma_start(
            out=x1t_r[:, :, qs],
            in_=x1td[:, qs].rearrange("(j p) w -> p j w", p=128))
        nc.gpsimd.dma_start(
            out=x2r_s[:, 4 * q:4 * (q + 1), :],
            in_=x2d[qs, :].rearrange("(t p) c -> p t c", p=128))

    # ---- constants / params (all bf16 from host) ---------------------------
    proj_s = persist.tile([128, NJ, K2], BF16)
    nc.sync.dma_start(out=proj_s, in_=projd.rearrange("(j p) k -> p j k", p=128))
    wmix_s = persist.tile([K2, C], BF16)
    nc.sync.dma_start(out=wmix_s, in_=wmixd)
    mmat_s = persist.tile([K, HPC, K], BF16)
    nc.sync.dma_start(out=mmat_s, in_=mmatd)
    ncolsum_s = persist.tile([1, K2], BF16)     # NEGATED colsum (host)
    nc.sync.dma_start(out=ncolsum_s, in_=colsumd)
    pbiasr_s = None
    if with_pbias:
        pbiasr_s = persist.tile([1, K2], BF16)
        nc.sync.dma_start(out=pbiasr_s, in_=pbiasrd)

    eps_s = persist.tile([128, 1], FP32)
    nc.vector.memset(eps_s, EPS)
    oneshalf = persist.tile([128, 1], BF16)
    nc.vector.memset(oneshalf, 1.0 / C)
    ident = persist.tile([128, 128], BF16)
    nc.vector.memset(ident, 0.0)
    make_identity(nc, ident, nomemset=True)
    # PE warm-up: data-independent matmuls while input DMA streams, so the
    # tensor engine p-state ramps before real work arrives.
    warm = ps_mo.tile([128, 512], FP32, tag="mo")
    for i in range(16):
        nc.tensor.matmul(warm[:, :128], lhsT=ident, rhs=ident,
                         start=True, stop=True)

    # ---- persistent activations --------------------------------------------
    rstd2_s = persist.tile([128, NT], FP32)     # per-row 1/std of x2
    mu2_s = persist.tile([128, NT], BF16)       # per-row NEGATED mean of x2
    rstd1_s = persist.tile([128, NT], FP32)     # per-row 1/std of x1
    sc12_s = persist.tile([128, NT], FP32)      # rstd1*rstd2 per row
    p1s_s = persist.tile([128, NT, K2], BF16)   # rstd1*rstd2 * p1u (rowwise)
    p1T_s = persist.tile([K2, W], BF16)         # p1^T UNSCALED (p1u^T)
    ft_s = persist.tile([K2, C], BF16)          # F^T
    f_s = persist.tile([128, NJ, K2], BF16)     # F (c on partitions)
    g_s = persist.tile([K, HPC, K], BF16)       # per-head Gram
    ht_bd_s = persist.tile([K2, K2], BF16)      # block-diag H^T = (M G)^T
    whm_s = persist.tile([K2, C], BF16)         # Whm = H_bd @ Wmix_slice
    if with_pbias:
        std2_s = persist.tile([128, NT], BF16)  # per-row std of x2
        s1_s = persist.tile([1, K2], BF16)      # column sums of p1

    def p1s_pairs(q):
        # paired PE-transposes + 0-stride-broadcast scale for quad q
        for tp in range(2):
            w0 = 4 * q + 2 * tp
            ps2 = ps_tp.tile([128, 2, 128], BF16, tag="tp")
            for i in range(2):
                nc.tensor.transpose(
                    ps2[:, i, :],
                    p1T_s[:, (w0 + i) * 128:(w0 + i + 1) * 128], ident)
            nc.vector.tensor_mul(p1s_s[:, w0:w0 + 2, :], ps2,
                                 _free_bcast(sc12_s[:, w0:w0 + 2], K2))

    facc = ps_acc.tile([128, 512], FP32, tag="facc")

    def wrow_f_partials(q):
        # F^T += p1s_t^T @ x2_t  and the folded mean-removal rank-1:
        # F^T += p1s_t^T @ ((-mu2_t) (x) ones) via 0-stride broadcast rhs.
        for t in range(4 * q, 4 * q + 4):
            nc.tensor.matmul(facc, lhsT=p1s_s[:, t, :], rhs=x2r_s[:, t, :],
                             start=(t == 0), stop=False)
        for t in range(4 * q, 4 * q + 4):
            nc.tensor.matmul(facc, lhsT=p1s_s[:, t, :],
                             rhs=_free_bcast(mu2_s[:, t:t + 1], 512),
                             start=False, stop=(t == NT - 1))

    # ========================================================================
    # Front end, pipelined per w-quad.
    # ========================================================================
    for q in range(NQ):
        qs = slice(q * 512, (q + 1) * 512)
        q4 = slice(4 * q, 4 * (q + 1))

        # ---- x1 squares on scalar (feed ssq matmuls) ----------------------
        sqt = sqpool.tile([128, NJ, 512], BF16, tag="sq")
        for j in range(NJ):
            nc.scalar.activation(sqt[:, j, :], x1t_r[:, j, qs], AF.Square)

        # ---- x2 row stats: bn per tile, tail ops batched over the quad ----
        mv = spool.tile([128, 4, 2], FP32, tag="mv")
        for t in range(4):
            stats = spool.tile([128, 6], FP32, tag="bst")
            nc.vector.bn_stats(stats, x2r_s[:, 4 * q + t, :])
            nc.vector.bn_aggr(mv[:, t, :], stats)
        std2q = spool.tile([128, 4], FP32, tag="stdq")
        nc.scalar.activation(std2q, mv[:, :, 1], AF.Sqrt, bias=eps_s, scale=1.0)
        nc.vector.reciprocal(rstd2_s[:, q4], std2q)
        nc.gpsimd.tensor_scalar_mul(mu2_s[:, q4], mv[:, :, 0], -1.0)
        if with_pbias:
            nc.gpsimd.tensor_copy(out=std2_s[:, q4], in_=std2q)

        # ---- PE: projection mains + mu stats (need only x1t) --------------
        pt = ps_mm.tile([128, 512], FP32, tag="mm")
        st_ps = ps_mo.tile([128, 512], FP32, tag="mo")
        mu_psv = st_ps[0:1, :]
        ssq_psv = st_ps[32:33, :]
        for j in range(NJ):
            nc.tensor.matmul(pt, lhsT=proj_s[:, j, :], rhs=x1t_r[:, j, qs],
                             start=(j == 0), stop=False)
        for j in range(NJ):
            nc.tensor.matmul(mu_psv, lhsT=oneshalf, rhs=x1t_r[:, j, qs],
                             start=(j == 0), stop=(j == NJ - 1))
        # filler: prev-quad transposes/partials while squares catch up
        if q > 0:
            p1s_pairs(q - 1)
            if q > 1:
                wrow_f_partials(q - 2)
        for j in range(NJ):
            nc.tensor.matmul(ssq_psv, lhsT=oneshalf, rhs=sqt[:, j, :],
                             start=(j == 0), stop=(j == NJ - 1))
        strow = rowpool.tile([33, 512], BF16, tag="strow")
        nc.scalar.copy(out=strow, in_=st_ps[0:33, :])

        # ---- close projection: += (-colsum) (x) mu_row --------------------
        nc.tensor.matmul(pt, lhsT=ncolsum_s, rhs=strow[0:1, :],
                         start=False, stop=not with_pbias)

        # ---- merged stat transposes: [33,128] -> [128,33] columns ---------
        stc = ps_st.tile([128, 4, 34], BF16, tag="st")
        for t in range(4):
            nc.tensor.transpose(stc[:, t, 0:33],
                                strow[:, t * 128:(t + 1) * 128],
                                ident[0:33, 0:33])
        stq = spool.tile([128, 4, 34], BF16, tag="stq")
        nc.vector.tensor_copy(out=stq, in_=stc)
        musq = spool.tile([128, 4], FP32, tag="musq")
        nc.gpsimd.tensor_mul(musq, stq[:, :, 0], stq[:, :, 0])
        varq = spool.tile([128, 4], FP32, tag="varq")
        nc.gpsimd.tensor_sub(varq, stq[:, :, 32], musq)
        std1q = spool.tile([128, 4], FP32, tag="std1")
        nc.scalar.activation(std1q, varq, AF.Sqrt, bias=eps_s, scale=1.0)
        nc.vector.reciprocal(rstd1_s[:, q4], std1q)
        nc.gpsimd.tensor_mul(sc12_s[:, q4], rstd1_s[:, q4], rstd2_s[:, q4])

        if with_pbias:
            # p1 = rstd1*p1u + 1 (x) pbias => fold as p1u += std1 (x) pbias
            std1q_b = spool.tile([128, 4], BF16, tag="s1qb")
            nc.gpsimd.tensor_copy(out=std1q_b, in_=std1q)
            s1r_ps = ps_st.tile([1, 512], BF16, tag="s1rp")
            for t in range(4):
                nc.tensor.transpose(s1r_ps[:, t * 128:(t + 1) * 128],
                                    std1q_b[:, t:t + 1], ident)
            std1row = rowpool.tile([1, 512], BF16, tag="s1r")
            nc.gpsimd.tensor_copy(out=std1row, in_=s1r_ps)
            nc.tensor.matmul(pt, lhsT=pbiasr_s, rhs=std1row,
                             start=False, stop=True)

        nc.vector.tensor_copy(out=p1T_s[:, qs], in_=pt)

    p1s_pairs(NQ - 1)
    wrow_f_partials(NQ - 2)
    wrow_f_partials(NQ - 1)

    # ========================================================================
    # Gram tail
    # ========================================================================
    nc.vector.tensor_copy(out=ft_s[:, 0:256], in_=facc[:, 0:256])
    nc.scalar.copy(out=ft_s[:, 256:512], in_=facc[:, 256:512])

    # F = PE-transpose of F^T (c on partitions), via tp pair tiles
    for jp in range(2):
        fjp = ps_tp.tile([128, 2, 128], BF16, tag="tp")
        for i in range(2):
            j = 2 * jp + i
            nc.tensor.transpose(fjp[:, i, :],
                                ft_s[:, j * 128:(j + 1) * 128], ident)
        if jp == 0:
            nc.vector.tensor_copy(out=f_s[:, 0:2, :], in_=fjp)
        else:
            nc.scalar.copy(out=f_s[:, 2:4, :], in_=fjp)

    # s1 = column sums of p1 = std2^T @ p1s (beta rank-1 term in G)
    if with_pbias:
        sp = ps_mm.tile([128, 512], FP32, tag="mm")
        spv = sp[:1, :K2]
        for t in range(NT):
            nc.tensor.matmul(spv, lhsT=std2_s[:, t:t + 1], rhs=p1s_s[:, t, :],
                             start=(t == 0), stop=(t == NT - 1))
        nc.vector.tensor_copy(out=s1_s, in_=spv)

    # Full P'^T @ F product; per-head Grams are its diagonal 64x64 blocks.
    gp = ps_mm.tile([128, 512], FP32, tag="mm")
    gpv = gp[:, :K2]
    for j in range(NJ):
        nc.tensor.matmul(gpv, lhsT=proj_s[:, j, :], rhs=f_s[:, j, :],
                         start=(j == 0),
                         stop=(j == NJ - 1) and not with_pbias)
    if with_pbias:
        # += pbias (x) s1 ; diagonal blocks get pbias_h (x) s1_h
        nc.tensor.matmul(gpv, lhsT=pbiasr_s, rhs=s1_s, start=False, stop=True)
    for h in range(HPC):
        nc.vector.tensor_copy(out=g_s[:, h, :],
                              in_=gpv[h * K:(h + 1) * K, h * K:(h + 1) * K])

    # H^T_h = G_h^T @ M_h (M symmetric); assemble block-diag H^T
    hp = ps_mm.tile([128, 512], FP32, tag="mm")
    for h in range(HPC):
        nc.tensor.matmul(hp[h * K:(h + 1) * K, :K], lhsT=g_s[:, h, :],
                         rhs=mmat_s[:, h, :])
    nc.vector.memset(ht_bd_s, 0.0)
    for h in range(HPC):
        nc.vector.tensor_copy(out=ht_bd_s[h * K:(h + 1) * K, h * K:(h + 1) * K],
                              in_=hp[h * K:(h + 1) * K, :K])

    # Whm = H_bd @ Wmix_slice  (lhsT = H^T_bd)
    whp = ps_mm.tile([128, 512], FP32, tag="mm")
    nc.tensor.matmul(whp, lhsT=ht_bd_s, rhs=wmix_s)
    nc.vector.tensor_copy(out=whm_s, in_=whp)

    # out = rstd1 * (p1u @ Whm), evacuated with per-partition rstd1 scale
    for h in range(2 * NQ):
        hs = slice(h * 256, (h + 1) * 256)
        stage = outpool.tile([128, 2, C], BF16, tag="ostage")
        for t in range(2):
            w_t = h * 2 + t
            mo = ps_mo.tile([128, 512], FP32, tag="mo")
            nc.tensor.matmul(mo, lhsT=p1T_s[:, w_t * 128:(w_t + 1) * 128],
                             rhs=whm_s)
            if t % 2 == 0:
                nc.vector.tensor_scalar_mul(stage[:, t, :], mo,
                                            rstd1_s[:, w_t:w_t + 1])
            else:
                nc.scalar.activation(stage[:, t, :], mo, AF.Copy,
                                     scale=rstd1_s[:, w_t:w_t + 1])
        nc.sync.dma_start(
            out=outd[hs, :].rearrange("(t p) c -> p t c", p=128),
            in_=stage)


_PROGRAM_CACHE = {}


def _get_program(with_pbias: bool):
    key = ("nc", with_pbias)
    if key in _PROGRAM_CACHE:
        return _PROGRAM_CACHE[key]
    nc = bacc.Bacc("TRN2", debug=False, num_devices=NCORES)
    x1td = nc.dram_tensor("x1t", [C, W], BF16, kind="ExternalInput").ap()
    x2d = nc.dram_tensor("x2", [W, C], BF16, kind="ExternalInput").ap()
    projd = nc.dram_tensor("proj", [C, K2], BF16, kind="ExternalInput").ap()
    mmatd = nc.dram_tensor("mmat", [K, HPC, K], BF16, kind="ExternalInput").ap()
    wmixd = nc.dram_tensor("wmix", [K2, C], BF16, kind="ExternalInput").ap()
    colsumd = nc.dram_tensor("colsum", [1, K2], BF16, kind="ExternalInput").ap()
    pbiasrd = None
    if with_pbias:
        pbiasrd = nc.dram_tensor("pbiasr", [1, K2], BF16, kind="ExternalInput").ap()
    outd = nc.dram_tensor("out", [W, C], BF16, kind="ExternalOutput").ap()
    with tile.TileContext(nc) as tc:
        with ExitStack() as ctx:
            _body(ctx, tc, x1td, x2d, projd, mmatd, wmixd, colsumd, outd,
                  pbiasrd)
    nc.compile()
    _PROGRAM_CACHE[key] = nc
    return nc


def _host_prep(inputs):
    x1 = np.asarray(inputs["x1"], np.float32)
    x2 = np.ascontiguousarray(np.asarray(inputs["x2"], np.float32))
    gamma = np.asarray(inputs["gamma"], np.float32)
    beta = np.asarray(inputs["beta"], np.float32)
    proj = np.asarray(inputs["proj_nck"], np.float32)
    halves = np.asarray(inputs["halves"], np.float32)
    diagonals = np.asarray(inputs["diagonals"], np.float32)
    wmix = np.asarray(inputs["W_mixer"], np.float32)

    iu0, iu1 = np.triu_indices(K, k=1)
    m = np.zeros((N, K, K), np.float32)
    m[:, iu0, iu1] = halves
    m = m + np.swapaxes(m, -1, -2)
    d = np.arange(K)
    m[:, d, d] = diagonals

    pgam = proj * gamma[None, :, None]          # gamma folded into projection
    with_pbias = bool(np.any(beta))
    pbias = np.einsum("c,nck->nk", beta, proj) if with_pbias else None

    x1t = [np.ascontiguousarray(x1[b].T).astype(NPBF16) for b in range(B)]
    x2b = [x2[b].astype(NPBF16) for b in range(B)]

    in_maps = []
    for core in range(NCORES):
        b, hg = divmod(core, NCORES // B)
        h0 = HPC * hg
        proj_core = np.ascontiguousarray(
            np.concatenate([pgam[h0 + i] for i in range(HPC)], axis=1))
        proj_bf = proj_core.astype(NPBF16)
        ncolsum = -proj_bf.astype(np.float32).sum(axis=0)
        im = {
            "x1t": x1t[b],
            "x2": x2b[b],
            "proj": proj_bf,
            "mmat": np.ascontiguousarray(
                np.stack([m[h0 + i] for i in range(HPC)], axis=1)).astype(NPBF16),
            "wmix": np.ascontiguousarray(
                wmix[:, K2 * hg:K2 * (hg + 1)].T).astype(NPBF16),
            "colsum": np.ascontiguousarray(ncolsum[None, :]).astype(NPBF16),
        }
        if with_pbias:
            pb = np.concatenate([pbias[h0 + i] for i in range(HPC)])
            im["pbiasr"] = np.ascontiguousarray(pb[None, :]).astype(NPBF16)
        in_maps.append(im)
    return in_maps, with_pbias


def kernel(**inputs) -> np.ndarray:
    in_maps, with_pbias = _host_prep(inputs)
    nc = _get_program(with_pbias)
    res = run_bass_kernel_spmd(nc, in_maps, core_ids=list(range(NCORES)))
    out = np.zeros((B, W, C), np.float32)
    for core in range(NCORES):
        b = core // (NCORES // B)
        out[b] += res.results[core]["out"].astype(np.float32)
    out += np.asarray(inputs["b_mixer"], np.float32)[None, None, :]
    return out


# revision 23
# speedup vs baseline: 1.1992x; 1.1524x over previous
"""Trainium2 Bass kernel for nn_MetricSelfAttention.

Math: the reference's softmax is dead code, so
    nudged = (p1 @ M @ p2^T) @ p1
reassociates to
    nudged = p1 @ (M @ (p2^T @ p1))        (per-head 64x64 Gram matrix G)
collapsing the O(W^2) attention matrices entirely, and the mixer folds to
    out = p1u @ Whm,   Whm = H_bd @ Wmix_slice   (precomputed once on-chip)
so the back end is a single matmul per w-tile straight from p1^T.

Sharding: 8 cores = 2 batches x 4 head-pairs.  Core (b, hg) computes heads
{2hg, 2hg+1} of batch b and the partial output; the host sums the 4 partials
per batch and adds b_mixer.  x1 arrives pre-transposed, everything pre-cast
to bf16 on the host (layout/dtype prep only).

LayerNorm folding (no normalized tensors are materialized):
  - gamma folds into the projection on the host; nonzero beta enters as
    rank-1 bias matmuls; omitted entirely when beta == 0.
  - x1 row stats come from x1^T via PE ones-matmuls producing rows at
    partitions 0/32 of one PSUM bank, which are PE-transposed to columns
    ([33,128] transposes) so all variance/sqrt/reciprocal arithmetic runs
    128-lane parallel.  Row-form stats never touch DVE single-lane ops.
  - rstd1 never exists in row form: p1^T stays UNSCALED; rstd1*rstd2 scales
    the transposed p1s columns (0-stride-broadcast paired multiply), and
    rstd1 alone scales the output-stage PSUM evacuation (per-partition).
    Mean-subtraction enters the projection as one rank-1 matmul
    (-colsum) (x) mu_row with colsum negated on the host.
  - x2 is never normalized or transposed:
      F := x2n^T @ p1  ==  x2^T @ p1s - 1 (x) (mu2^T @ p1s),  p1s = rstd12*p1u

All tensors flow as bf16 (PSUM accumulation in fp32); partial outputs are
written bf16 and summed in fp32 on the host.
"""

from contextlib import ExitStack

import numpy as np
import ml_dtypes

import concourse.bacc as bacc
import concourse.bass as bass
import concourse.tile as tile
from concourse import mybir
from concourse.bass_utils import run_bass_kernel_spmd
from concourse.masks import make_identity

B, W, C, N, K = 2, 2048, 512, 8, 64
NCORES = 8
HPC = 2          # heads per core
K2 = HPC * K     # 128 channels per core
EPS = 1e-5
FP32 = mybir.dt.float32
BF16 = mybir.dt.bfloat16
NPBF16 = ml_dtypes.bfloat16
AF = mybir.ActivationFunctionType

NT = W // 128    # 16 w-tiles
NQ = W // 512    # 4 w-quads
NJ = C // 128    # 4 c-chunks


def _free_bcast(ap2d, n):
    """[128, m] AP -> [128, m, n] view with 0-stride innermost dim."""
    return bass.AP(tensor=ap2d.tensor, offset=ap2d.offset,
                   ap=list(ap2d.ap) + [[0, n]])


def _body(ctx: ExitStack, tc: tile.TileContext, x1td, x2d, projd, mmatd,
          wmixd, colsumd, outd, pbiasrd):
    nc = tc.nc
    with_pbias = pbiasrd is not None

    persist = ctx.enter_context(tc.tile_pool(name="persist", bufs=1))
    sqpool = ctx.enter_context(tc.tile_pool(name="sq", bufs=2))
    rowpool = ctx.enter_context(tc.tile_pool(name="rows", bufs=2))
    spool = ctx.enter_context(tc.tile_pool(name="stats", bufs=4))
    outpool = ctx.enter_context(tc.tile_pool(name="outstage", bufs=2))
    ps_tp = ctx.enter_context(tc.tile_pool(
        name="ps_tp", bufs=1 if with_pbias else 2, space="PSUM"))
    ps_acc = ctx.enter_context(tc.tile_pool(name="ps_acc", bufs=1, space="PSUM"))
    ps_st = ctx.enter_context(tc.tile_pool(name="ps_st", bufs=1, space="PSUM"))
    ps_mm = ctx.enter_context(tc.tile_pool(name="ps_mm", bufs=2, space="PSUM"))
    ps_mo = ctx.enter_context(tc.tile_pool(name="ps_mo", bufs=2, space="PSUM"))

    # ---- persistent activations (declared first; loads issued immediately) -
    x1t_r = persist.tile([128, NJ, W], BF16)    # x1^T
    x2r_s = persist.tile([128, NT, C], BF16)    # raw x2
    for q in range(NQ):
        qs = slice(q * 512, (q + 1) * 512)
        nc.gpsimd.dma_start(
            out=x1t_r[:, :, qs],
            in_=x1td[:, qs].rearrange("(j p) w -> p j w", p=128))
        nc.gpsimd.dma_start(
            out=x2r_s[:, 4 * q:4 * (q + 1), :],
            in_=x2d[qs, :].rearrange("(t p) c -> p t c", p=128))

    # ---- constants / params (all bf16 from host) ---------------------------
    proj_s = persist.tile([128, NJ, K2], BF16)
    nc.sync.dma_start(out=proj_s, in_=projd.rearrange("(j p) k -> p j k", p=128))
    wmix_s = persist.tile([K2, C], BF16)
    nc.sync.dma_start(out=wmix_s, in_=wmixd)
    mmat_s = persist.tile([K, HPC, K], BF16)
    nc.sync.dma_start(out=mmat_s, in_=mmatd)
    ncolsum_s = persist.tile([1, K2], BF16)     # NEGATED colsum (host)
    nc.sync.dma_start(out=ncolsum_s, in_=colsumd)
    pbiasr_s = None
    if with_pbias:
        pbiasr_s = persist.tile([1, K2], BF16)
        nc.sync.dma_start(out=pbiasr_s, in_=pbiasrd)

    eps_s = persist.tile([128, 1], FP32)
    nc.vector.memset(eps_s, EPS)
    oneshalf = persist.tile([128, 1], BF16)
    nc.vector.memset(oneshalf, 1.0 / C)
    ident = persist.tile([128, 128], BF16)
    nc.vector.memset(ident, 0.0)
    make_identity(nc, ident, nomemset=True)

    # ---- persistent activations --------------------------------------------
    rstd2_s = persist.tile([128, NT], FP32)     # per-row 1/std of x2
    mu2_s = persist.tile([128, NT], BF16)       # per-row NEGATED mean of x2
    rstd1_s = persist.tile([128, NT], FP32)     # per-row 1/std of x1
    sc12_s = persist.tile([128, NT], FP32)      # rstd1*rstd2 per row
    p1s_s = persist.tile([128, NT, K2], BF16)   # rstd1*rstd2 * p1u (rowwise)
    p1T_s = persist.tile([K2, W], BF16)         # p1^T UNSCALED (p1u^T)
    ft_s = persist.tile([K2, C], BF16)          # F^T
    f_s = persist.tile([128, NJ, K2], BF16)     # F (c on partitions)
    g_s = persist.tile([K, HPC, K], BF16)       # per-head Gram
    ht_bd_s = persist.tile([K2, K2], BF16)      # block-diag H^T = (M G)^T
    whm_s = persist.tile([K2, C], BF16)         # Whm = H_bd @ Wmix_slice
    if with_pbias:
        std2_s = persist.tile([128, NT], BF16)  # per-row std of x2
        s1_s = persist.tile([1, K2], BF16)      # column sums of p1

    def p1s_pairs(q):
        # paired PE-transposes + 0-stride-broadcast scale for quad q
        for tp in range(2):
            w0 = 4 * q + 2 * tp
            ps2 = ps_tp.tile([128, 2, 128], BF16, tag="tp")
            for i in range(2):
                nc.tensor.transpose(
                    ps2[:, i, :],
                    p1T_s[:, (w0 + i) * 128:(w0 + i + 1) * 128], ident)
            nc.vector.tensor_mul(p1s_s[:, w0:w0 + 2, :], ps2,
                                 _free_bcast(sc12_s[:, w0:w0 + 2], K2))

    facc = ps_acc.tile([128, 512], FP32, tag="facc")

    def wrow_f_partials(q):
        # F^T += p1s_t^T @ x2_t  and the folded mean-removal rank-1:
        # F^T += p1s_t^T @ ((-mu2_t) (x) ones) via 0-stride broadcast rhs.
        for t in range(4 * q, 4 * q + 4):
            nc.tensor.matmul(facc, lhsT=p1s_s[:, t, :], rhs=x2r_s[:, t, :],
                             start=(t == 0), stop=False)
        for t in range(4 * q, 4 * q + 4):
            nc.tensor.matmul(facc, lhsT=p1s_s[:, t, :],
                             rhs=_free_bcast(mu2_s[:, t:t + 1], 512),
                             start=False, stop=(t == NT - 1))

    # ========================================================================
    # Front end, pipelined per w-quad.
    # ========================================================================
    for q in range(NQ):
        qs = slice(q * 512, (q + 1) * 512)
        q4 = slice(4 * q, 4 * (q + 1))

        # ---- x1 squares on scalar (feed ssq matmuls) ----------------------
        sqt = sqpool.tile([128, NJ, 512], BF16, tag="sq")
        for j in range(NJ):
            nc.scalar.activation(sqt[:, j, :], x1t_r[:, j, qs], AF.Square)

        # ---- x2 row stats: bn per tile, tail ops batched over the quad ----
        mv = spool.tile([128, 4, 2], FP32, tag="mv")
        for t in range(4):
            stats = spool.tile([128, 6], FP32, tag="bst")
            nc.vector.bn_stats(stats, x2r_s[:, 4 * q + t, :])
            nc.vector.bn_aggr(mv[:, t, :], stats)
        std2q = spool.tile([128, 4], FP32, tag="stdq")
        nc.scalar.activation(std2q, mv[:, :, 1], AF.Sqrt, bias=eps_s, scale=1.0)
        nc.vector.reciprocal(rstd2_s[:, q4], std2q)
        nc.gpsimd.tensor_scalar_mul(mu2_s[:, q4], mv[:, :, 0], -1.0)
        if with_pbias:
            nc.gpsimd.tensor_copy(out=std2_s[:, q4], in_=std2q)

        # ---- PE: projection mains + mu stats (need only x1t) --------------
        pt = ps_mm.tile([128, 512], FP32, tag="mm")
        st_ps = ps_mo.tile([128, 512], FP32, tag="mo")
        mu_psv = st_ps[0:1, :]
        ssq_psv = st_ps[32:33, :]
        for j in range(NJ):
            nc.tensor.matmul(pt, lhsT=proj_s[:, j, :], rhs=x1t_r[:, j, qs],
                             start=(j == 0), stop=False)
        for j in range(NJ):
            nc.tensor.matmul(mu_psv, lhsT=oneshalf, rhs=x1t_r[:, j, qs],
                             start=(j == 0), stop=(j == NJ - 1))
        # filler: prev-quad transposes/partials while squares catch up
        if q > 0:
            p1s_pairs(q - 1)
            if q > 1:
                wrow_f_partials(q - 2)
        for j in range(NJ):
            nc.tensor.matmul(ssq_psv, lhsT=oneshalf, rhs=sqt[:, j, :],
                             start=(j == 0), stop=(j == NJ - 1))
        strow = rowpool.tile([33, 512], BF16, tag="strow")
        nc.scalar.copy(out=strow, in_=st_ps[0:33, :])

        # ---- close projection: += (-colsum) (x) mu_row --------------------
        nc.tensor.matmul(pt, lhsT=ncolsum_s, rhs=strow[0:1, :],
                         start=False, stop=not with_pbias)

        # ---- merged stat transposes: [33,128] -> [128,33] columns ---------
        stc = ps_st.tile([128, 4, 34], BF16, tag="st")
        for t in range(4):
            nc.tensor.transpose(stc[:, t, 0:33],
                                strow[:, t * 128:(t + 1) * 128],
                                ident[0:33, 0:33])
        stq = spool.tile([128, 4, 34], BF16, tag="stq")
        nc.vector.tensor_copy(out=stq, in_=stc)
        musq = spool.tile([128, 4], FP32, tag="musq")
        nc.gpsimd.tensor_mul(musq, stq[:, :, 0], stq[:, :, 0])
        varq = spool.tile([128, 4], FP32, tag="varq")
        nc.gpsimd.tensor_sub(varq, stq[:, :, 32], musq)
        std1q = spool.tile([128, 4], FP32, tag="std1")
        nc.scalar.activation(std1q, varq, AF.Sqrt, bias=eps_s, scale=1.0)
        nc.vector.reciprocal(rstd1_s[:, q4], std1q)
        nc.gpsimd.tensor_mul(sc12_s[:, q4], rstd1_s[:, q4], rstd2_s[:, q4])

        if with_pbias:
            # p1 = rstd1*p1u + 1 (x) pbias => fold as p1u += std1 (x) pbias
            std1q_b = spool.tile([128, 4], BF16, tag="s1qb")
            nc.gpsimd.tensor_copy(out=std1q_b, in_=std1q)
            s1r_ps = ps_st.tile([1, 512], BF16, tag="s1rp")
            for t in range(4):
                nc.tensor.transpose(s1r_ps[:, t * 128:(t + 1) * 128],
                                    std1q_b[:, t:t + 1], ident)
            std1row = rowpool.tile([1, 512], BF16, tag="s1r")
            nc.gpsimd.tensor_copy(out=std1row, in_=s1r_ps)
            nc.tensor.matmul(pt, lhsT=pbiasr_s, rhs=std1row,
                             start=False, stop=True)

        nc.vector.tensor_copy(out=p1T_s[:, qs], in_=pt)

    p1s_pairs(NQ - 1)
    wrow_f_partials(NQ - 2)
    wrow_f_partials(NQ - 1)

    # ========================================================================
    # Gram tail
    # ========================================================================
    nc.vector.tensor_copy(out=ft_s[:, 0:256], in_=facc[:, 0:256])
    nc.scalar.copy(out=ft_s[:, 256:512], in_=facc[:, 256:512])

    # F = PE-transpose of F^T (c on partitions), via tp pair tiles
    for jp in range(2):
        fjp = ps_tp.tile([128, 2, 128], BF16, tag="tp")
        for i in range(2):
            j = 2 * jp + i
            nc.tensor.transpose(fjp[:, i, :],
                                ft_s[:, j * 128:(j + 1) * 128], ident)
        if jp == 0:
            nc.vector.tensor_copy(out=f_s[:, 0:2, :], in_=fjp)
        else:
            nc.scalar.copy(out=f_s[:, 2:4, :], in_=fjp)

    # s1 = column sums of p1 = std2^T @ p1s (beta rank-1 term in G)
    if with_pbias:
        sp = ps_mm.tile([128, 512], FP32, tag="mm")
        spv = sp[:1, :K2]
        for t in range(NT):
            nc.tensor.matmul(spv, lhsT=std2_s[:, t:t + 1], rhs=p1s_s[:, t, :],
                             start=(t == 0), stop=(t == NT - 1))
        nc.vector.tensor_copy(out=s1_s, in_=spv)

    # Full P'^T @ F product; per-head Grams are its diagonal 64x64 blocks.
    gp = ps_mm.tile([128, 512], FP32, tag="mm")
    gpv = gp[:, :K2]
    for j in range(NJ):
        nc.tensor.matmul(gpv, lhsT=proj_s[:, j, :], rhs=f_s[:, j, :],
                         start=(j == 0),
                         stop=(j == NJ - 1) and not with_pbias)
    if with_pbias:
        # += pbias (x) s1 ; diagonal blocks get pbias_h (x) s1_h
        nc.tensor.matmul(gpv, lhsT=pbiasr_s, rhs=s1_s, start=False, stop=True)
    for h in range(HPC):
        nc.vector.tensor_copy(out=g_s[:, h, :],
                              in_=gpv[h * K:(h + 1) * K, h * K:(h + 1) * K])

    # H^T_h = G_h^T @ M_h (M symmetric); assemble block-diag H^T
    hp = ps_mm.tile([128, 512], FP32, tag="mm")
    for h in range(HPC):
        nc.tensor.matmul(hp[h * K:(h + 1) * K, :K], lhsT=g_s[:, h, :],
                         rhs=mmat_s[:, h, :])
    nc.vector.memset(ht_bd_s, 0.0)
    for h in range(HPC):
        nc.vector.tensor_copy(out=ht_bd_s[h * K:(h + 1) * K, h * K:(h + 1) * K],
                              in_=hp[h * K:(h + 1) * K, :K])

    # Whm = H_bd @ Wmix_slice  (lhsT = H^T_bd)
    whp = ps_mm.tile([128, 512], FP32, tag="mm")
    nc.tensor.matmul(whp, lhsT=ht_bd_s, rhs=wmix_s)
    nc.vector.tensor_copy(out=whm_s, in_=whp)

    # out = rstd1 * (p1u @ Whm), evacuated with per-partition rstd1 scale
    for h in range(2 * NQ):
        hs = slice(h * 256, (h + 1) * 256)
        stage = outpool.tile([128, 2, C], BF16, tag="ostage")
        for t in range(2):
            w_t = h * 2 + t
            mo = ps_mo.tile([128, 512], FP32, tag="mo")
            nc.tensor.matmul(mo, lhsT=p1T_s[:, w_t * 128:(w_t + 1) * 128],
                             rhs=whm_s)
            if t % 2 == 0:
                nc.vector.tensor_scalar_mul(stage[:, t, :], mo,
                                            rstd1_s[:, w_t:w_t + 1])
            else:
                nc.scalar.activation(stage[:, t, :], mo, AF.Copy,
                                     scale=rstd1_s[:, w_t:w_t + 1])
        nc.sync.dma_start(
            out=outd[hs, :].rearrange("(t p) c -> p t c", p=128),
            in_=stage)


_PROGRAM_CACHE = {}


def _get_program(with_pbias: bool):
    key = ("nc", with_pbias)
    if key in _PROGRAM_CACHE:
        return _PROGRAM_CACHE[key]
    nc = bacc.Bacc("TRN2", debug=False, num_devices=NCORES)
    x1td = nc.dram_tensor("x1t", [C, W], BF16, kind="ExternalInput").ap()
    x2d = nc.dram_tensor("x2", [W, C], BF16, kind="ExternalInput").ap()
    projd = nc.dram_tensor("proj", [C, K2], BF16, kind="ExternalInput").ap()
    mmatd = nc.dram_tensor("mmat", [K, HPC, K], BF16, kind="ExternalInput").ap()
    wmixd = nc.dram_tensor("wmix", [K2, C], BF16, kind="ExternalInput").ap()
    colsumd = nc.dram_tensor("colsum", [1, K2], BF16, kind="ExternalInput").ap()
    pbiasrd = None
    if with_pbias:
        pbiasrd = nc.dram_tensor("pbiasr", [1, K2], BF16, kind="ExternalInput").ap()
    outd = nc.dram_tensor("out", [W, C], BF16, kind="ExternalOutput").ap()
    with tile.TileContext(nc) as tc:
        with ExitStack() as ctx:
            _body(ctx, tc, x1td, x2d, projd, mmatd, wmixd, colsumd, outd,
                  pbiasrd)
    nc.compile()
    _PROGRAM_CACHE[key] = nc
    return nc


def _host_prep(inputs):
    x1 = np.asarray(inputs["x1"], np.float32)
    x2 = np.ascontiguousarray(np.asarray(inputs["x2"], np.float32))
    gamma = np.asarray(inputs["gamma"], np.float32)
    beta = np.asarray(inputs["beta"], np.float32)
    proj = np.asarray(inputs["proj_nck"], np.float32)
    halves = np.asarray(inputs["halves"], np.float32)
    diagonals = np.asarray(inputs["diagonals"], np.float32)
    wmix = np.asarray(inputs["W_mixer"], np.float32)

    iu0, iu1 = np.triu_indices(K, k=1)
    m = np.zeros((N, K, K), np.float32)
    m[:, iu0, iu1] = halves
    m = m + np.swapaxes(m, -1, -2)
    d = np.arange(K)
    m[:, d, d] = diagonals

    pgam = proj * gamma[None, :, None]          # gamma folded into projection
    with_pbias = bool(np.any(beta))
    pbias = np.einsum("c,nck->nk", beta, proj) if with_pbias else None

    x1t = [np.ascontiguousarray(x1[b].T).astype(NPBF16) for b in range(B)]
    x2b = [x2[b].astype(NPBF16) for b in range(B)]

    in_maps = []
    for core in range(NCORES):
        b, hg = divmod(core, NCORES // B)
        h0 = HPC * hg
        proj_core = np.ascontiguousarray(
            np.concatenate([pgam[h0 + i] for i in range(HPC)], axis=1))
        proj_bf = proj_core.astype(NPBF16)
        ncolsum = -proj_bf.astype(np.float32).sum(axis=0)
        im = {
            "x1t": x1t[b],
            "x2": x2b[b],
            "proj": proj_bf,
            "mmat": np.ascontiguousarray(
                np.stack([m[h0 + i] for i in range(HPC)], axis=1)).astype(NPBF16),
            "wmix": np.ascontiguousarray(
                wmix[:, K2 * hg:K2 * (hg + 1)].T).astype(NPBF16),
            "colsum": np.ascontiguousarray(ncolsum[None, :]).astype(NPBF16),
        }
        if with_pbias:
            pb = np.concatenate([pbias[h0 + i] for i in range(HPC)])
            im["pbiasr"] = np.ascontiguousarray(pb[None, :]).astype(NPBF16)
        in_maps.append(im)
    return in_maps, with_pbias


def kernel(**inputs) -> np.ndarray:
    in_maps, with_pbias = _host_prep(inputs)
    nc = _get_program(with_pbias)
    res = run_bass_kernel_spmd(nc, in_maps, core_ids=list(range(NCORES)))
    out = np.zeros((B, W, C), np.float32)
    for core in range(NCORES):
        b = core // (NCORES // B)
        out[b] += res.results[core]["out"].astype(np.float32)
    out += np.asarray(inputs["b_mixer"], np.float32)[None, None, :]
    return out
